# revision 1
# baseline (speedup 1.0000x reference)
"""Trainium2 Bass kernel for the focal-modulation dense_cnn problem.

Math (per reference):
  fx = conv1x1(x, f_w, f_b);  q, gates = fx[:, :C], fx[:, C:]
  ctx = sum_l x_list[l] * gates[:, l]
  mod = conv1x1(ctx, h_w, h_b)
  y   = conv1x1(q * mod, proj_w, proj_b)
  out = layernorm_c(y) * ln_w + ln_b + x

Strategy: data-parallel over batch (16 -> 2 per core, 8 cores). On-chip
layout keeps channels on partitions as [96, 2, F] (c = p + 96*j), pixels on
the free dim, so all 1x1 convs are plain matmuls contracted over the two
96-row halves (+ a ones-row carrying biases). Per-pixel gate scalars are
broadcast across partitions by replicating the gate weight vector into a
[K, 96] stationary operand. Mean-centering of the LayerNorm is folded into
the proj weights host-side (pw' = pw - w_mu), so only the variance needs
an on-chip reduction (a 1/C-weighted ones-matmul over squares); 1/std is
exp(-0.5*ln(var+eps)) on the scalar engine. Matmuls run as float32r.
"""

import os
import sys

sys.path.insert(0, "/opt/trn_rl_repo")

import numpy as np

import bass_rust
import concourse.bass as bass
import concourse.mybir as mybir
import concourse.tile as tile
from concourse.bass_utils import run_bass_kernel_spmd
from concourse.vector_clock import ScopedClock

# ---------------------------------------------------------------------------
# Workaround: this walrus build accepts only one sem wait per instruction
# ("Too many sync wait commands"). (1) chain the Tile tail drain's waits;
# (2) post-pass that moves excess waits onto NoOps inserted just before the
# offending instruction on the same engine.


def _patched_drain_and_barrier(self, tick_clock, wait_clock):
    nc = self.nc
    drain_inst = nc.sync.drain()
    wait_clock.add_sem_waits(
        drain_inst.ins, ScopedClock({None: tick_clock.global_clock})
    )
    si = drain_inst.ins.sync_info
    if si is not None and len(si.on_wait) > 1:
        waits = list(si.on_wait)
        drain_inst.ins.sync_info = bass_rust.SyncInfo(
            on_wait=waits[:1], on_update=list(si.on_update)
        )
        for w in waits[1:]:
            d2 = nc.sync.drain()
            d2.ins.sync_info = bass_rust.SyncInfo(on_wait=[w], on_update=[])
    nc.all_engine_barrier()
    assert self.sems is not None
    popped = nc._tile_sem_poison_stack.pop()
    assert popped is self._sem_poison
    nc.clear_and_free_semaphores(list(self.sems.allocated().values()))
    nc.all_engine_barrier()


tile.TileContext._drain_and_barrier = _patched_drain_and_barrier

_WAIT_LIMIT = 1


def _split_excess_waits(nc):
    k = 0
    for f in nc.m.functions:
        for b in f.blocks:
            il = b.instructions
            new = []
            for inst in il:
                si = inst.sync_info
                if si is not None and len(si.on_wait) > _WAIT_LIMIT:
                    waits = list(si.on_wait)
                    excess, keep = waits[:-_WAIT_LIMIT], waits[-_WAIT_LIMIT:]
                    for w in excess:
                        nop = mybir.InstNoOp(name=f"wsplit-{k}",
                                             engine=inst.engine)
                        nop.sync_info = bass_rust.SyncInfo(on_wait=[w],
                                                           on_update=[])
                        new.append(nop)
                        k += 1
                    inst.sync_info = bass_rust.SyncInfo(
                        on_wait=keep, on_update=list(si.on_update))
                new.append(inst)
            il[:] = new
    return k
# ---------------------------------------------------------------------------

FP32 = mybir.dt.float32
BF16 = mybir.dt.bfloat16
F32R = mybir.dt.float32r
AF = mybir.ActivationFunctionType
OP = mybir.AluOpType

NCORES = 8
N_FULL, C, H, W, L = 16, 192, 128, 128, 3
HW = H * W
NS = N_FULL // NCORES          # batch per core
MAC = 1024                     # pixels per DMA macro-tile
F = 512                        # pixels per inner/PSUM tile
NMAC = HW // MAC
NF = MAC // F
EPS = 1e-6
STAGE = int(os.environ.get("KSTAGE", "9"))
PSA = int(os.environ.get("KPSA", "2"))
PSB = int(os.environ.get("KPSB", "2"))
PSC = int(os.environ.get("KPSC", "2"))
WRKB = int(os.environ.get("KWRK", "2"))
BF16G = os.environ.get("KBF16", "0") == "1"
VARB = int(os.environ.get("KVAR", "2"))
INPB = int(os.environ.get("KINP", "2"))

_prog_cache = {}


def _bc2(ap, n=2):
    """[P, F] -> [P, n, F] view with a step-0 middle dim (free broadcast)."""
    return bass.AP(tensor=ap.tensor, offset=ap.offset,
                   ap=[ap.ap[0], [0, n], ap.ap[1]])


def _build_program():
    nc = bass.Bass(trn_type="TRN2")

    d_x = nc.dram_tensor("x", [NS, 194, HW], F32R, kind="ExternalInput")
    d_xl = [nc.dram_tensor(f"xl{l}", [NS, C, HW],
                           FP32 if BF16G else F32R, kind="ExternalInput")
            for l in range(L)]
    d_fw = [nc.dram_tensor(f"fw{j}", [97, 192], F32R, kind="ExternalInput")
            for j in range(2)]
    d_wg = [nc.dram_tensor(f"wg{j}", [97, 288], F32R, kind="ExternalInput")
            for j in range(2)]
    d_hw = nc.dram_tensor("hw_t", [C, 192],
                          BF16 if BF16G else F32R, kind="ExternalInput")
    d_pj = nc.dram_tensor("pjc_t", [C, 192], F32R, kind="ExternalInput")
    d_lnw = nc.dram_tensor("lnw", [1, C], F32R, kind="ExternalInput")
    d_oc = nc.dram_tensor("oc", [96, 1], F32R, kind="ExternalInput")
    d_hb = nc.dram_tensor("hb", [96, 2], FP32, kind="ExternalInput")
    d_pbt = nc.dram_tensor("pbt", [96, 2], FP32, kind="ExternalInput")
    d_lnb = nc.dram_tensor("lnbc", [96, 2], FP32, kind="ExternalInput")
    d_out = nc.dram_tensor("out", [NS, C, HW], FP32, kind="ExternalOutput")

    # x: [NS, 194, HW] -> [NS][97, 2, HW] with aug channel c = p + 97*j
    vx = d_x[:, :, :].rearrange("n (j p) w -> n p j w", j=2)
    vxl = [t[:, :, :].rearrange("n (j p) w -> n p j w", j=2) for t in d_xl]
    vout = d_out[:, :, :].rearrange("n (j p) w -> n p j w", j=2)

    from contextlib import ExitStack
    with tile.TileContext(nc) as tc, ExitStack() as ctx:
        sing = ctx.enter_context(tc.tile_pool(name="sing", bufs=1))
        inp = ctx.enter_context(tc.tile_pool(name="inp", bufs=INPB))
        outp = ctx.enter_context(tc.tile_pool(name="outp", bufs=2))
        wrk = ctx.enter_context(tc.tile_pool(name="wrk", bufs=WRKB))
        rows = ctx.enter_context(tc.tile_pool(name="rows", bufs=2))
        psA = ctx.enter_context(tc.tile_pool(name="psA", bufs=PSA, space="PSUM"))
        psB = ctx.enter_context(tc.tile_pool(name="psB", bufs=PSB, space="PSUM"))
        psC = ctx.enter_context(tc.tile_pool(name="psC", bufs=PSC, space="PSUM"))

        # ---- load constants / weights (once) ----
        def loadw(dram, r0, r1, m, tag):
            t = sing.tile([r1 - r0, m], F32R, tag=tag, name=tag)
            nc.sync.dma_start(out=t, in_=dram[r0:r1, :])
            return t

        fw = [loadw(d_fw[j], 0, 97, 192, f"fw{j}") for j in range(2)]
        wg = [loadw(d_wg[j], 0, 97, 288, f"wg{j}") for j in range(2)]
        def loadw_t(dram, r0, r1, m, tag, dt):
            t = sing.tile([r1 - r0, m], dt, tag=tag, name=tag)
            nc.sync.dma_start(out=t, in_=dram[r0:r1, :])
            return t

        hwt = [loadw_t(d_hw, 96 * j, 96 * (j + 1), 192, f"hw{j}",
                       BF16 if BF16G else F32R) for j in range(2)]
        pjc = [loadw(d_pj, 96 * j, 96 * (j + 1), 192, f"pj{j}")
               for j in range(2)]
        lnw = loadw(d_lnw, 0, 1, C, "lnw")
        oc = loadw(d_oc, 0, 96, 1, "oc")

        def load_small(dram, shape, tag):
            t = sing.tile(shape, FP32, tag=tag, name=tag)
            nc.sync.dma_start(out=t, in_=dram[:, :])
            return t

        hb = load_small(d_hb, [96, 2], "hb")
        pbt = load_small(d_pbt, [96, 2], "pbt")
        lnb = load_small(d_lnb, [96, 2], "lnb")
        eps_t = sing.tile([1, 1], FP32, tag="eps")
        nc.vector.memset(eps_t, EPS)

        mm = nc.tensor.matmul

        # ---- main loop ----
        # Per-tile emission is staged so the in-order PE queue never stalls
        # on tail ops: A(i)=fx/G/gating, B(i-1)=deferred tail (var/A2/z1/out),
        # C(i)=h/xo/proj/sq.
        tiles = [(n, im, fi) for n in range(NS) for im in range(NMAC)
                 for fi in range(NF)]
        macs = {}
        st = {}

        def load_macro(n, im):
            o0 = im * MAC
            x_t = inp.tile([97, 2, MAC], F32R, tag="x", name="x_t")
            nc.sync.dma_start(out=x_t, in_=vx[n, :, :, o0:o0 + MAC])
            xl_t = []
            for l in range(L):
                t = inp.tile([96, 2, MAC], BF16 if BF16G else F32R,
                             tag=f"xl{l}", name=f"xl{l}")
                nc.sync.dma_start(out=t, in_=vxl[l][n, :, :, o0:o0 + MAC])
                xl_t.append(t)
            out_t = outp.tile([96, 2, MAC], FP32, tag="out", name="out_t")
            macs[(n, im)] = (x_t, xl_t, out_t)

        def stage_a(i):
            n, im, fi = tiles[i]
            if (n, im) not in macs:
                load_macro(n, im)
            x_t, xl_t, out_t = macs[(n, im)]
            s0 = fi * F
            sl = slice(s0, s0 + F)
            xj = [x_t[:, j, sl] for j in range(2)]
            fx = []
            for m in range(2):
                p = psA.tile([96, F], FP32, tag="pa", name=f"fx{m}")
                for j in range(2):
                    mm(p, fw[j][:, m * 96:(m + 1) * 96], xj[j],
                       start=(j == 0), stop=(j == 1))
                fx.append(p)
            t_l = []
            for l in range(L):
                G = psB.tile([96, F], FP32, tag="pb", name=f"G{l}")
                for j in range(2):
                    mm(G, wg[j][:, l * 96:(l + 1) * 96], xj[j],
                       start=(j == 0), stop=(j == 1))
                t = wrk.tile([96, 2, F], F32R, tag=f"t{l}", name=f"t{l}")
                for j in range(2):
                    nc.vector.tensor_tensor(t[:, j, :],
                                            xl_t[l][:, j, sl], G, OP.mult)
                t_l.append(t)
            st[i] = {"fx": fx, "t_l": t_l, "sl": sl, "x_t": x_t,
                     "out_t": out_t, "n": n, "im": im, "fi": fi}

        def stage_c(i):
            d = st[i]
            fx, t_l = d["fx"], d["t_l"]
            mod = []
            for m in range(2):
                p = psC.tile([96, F], FP32, tag="pc", name=f"mod{m}")
                k = 0
                for l in range(L):
                    for j in range(2):
                        mm(p, hwt[j][:, m * 96:(m + 1) * 96],
                           t_l[l][:, j, :], start=(k == 0), stop=(k == 5))
                        k += 1
                mod.append(p)
            msb = wrk.tile([96, 2, F], F32R, tag="msb", name="msb")
            for m in range(2):
                nc.scalar.activation(msb[:, m, :], mod[m], AF.Identity,
                                     bias=hb[:, m:m + 1])
            for m in range(2):
                nc.vector.tensor_tensor(msb[:, m, :], msb[:, m, :], fx[m],
                                        OP.mult)
            pjp = []
            for m in range(2):
                p = psA.tile([96, F], FP32, tag="pa", name=f"pj{m}")
                for j in range(2):
                    mm(p, pjc[j][:, m * 96:(m + 1) * 96], msb[:, j, :],
                       start=(j == 0), stop=(j == 1))
                pjp.append(p)
            pj_sb = wrk.tile([96, 2, F], FP32, tag="pjsb", name="pj_sb")
            for m in range(2):
                nc.scalar.activation(pj_sb[:, m, :], pjp[m], AF.Identity,
                                     bias=pbt[:, m:m + 1])
            sq = wrk.tile([96, 2, F], F32R, tag="sq", name="sq")
            for m in range(2):
                nc.scalar.activation(sq[:, m, :], pj_sb[:, m, :], AF.Square)
            d["pj_sb"] = pj_sb
            d["sq"] = sq

        def stage_b(i):
            d = st.pop(i)
            sq, pj_sb, sl = d["sq"], d["pj_sb"], d["sl"]
            x_t, out_t = d["x_t"], d["out_t"]
            var = psB.tile([1, F], FP32, tag="var", name="var", bufs=VARB)
            for j in range(2):
                mm(var, oc, sq[:, j, :], start=(j == 0), stop=(j == 1))
            lnv = rows.tile([1, F], FP32, tag="lnv", name="lnv")
            nc.scalar.activation(lnv, var, AF.Ln, bias=eps_t)
            istd = rows.tile([1, F], F32R, tag="istd", name="istd")
            nc.scalar.activation(istd, lnv, AF.Exp, scale=-0.5)
            z1 = wrk.tile([96, 2, F], FP32, tag="z1", name="z1")
            for m in range(2):
                a2 = psB.tile([96, F], FP32, tag="var", name=f"a2{m}",
                              bufs=VARB)
                mm(a2, lnw[0:1, m * 96:(m + 1) * 96], istd)
                nc.vector.tensor_tensor(z1[:, m, :], pj_sb[:, m, :], a2,
                                        OP.mult)
            for m in range(2):
                nc.gpsimd.tensor_scalar_add(z1[:, m, :], z1[:, m, :],
                                            lnb[:, m:m + 1])
            for m in range(2):
                nc.gpsimd.tensor_tensor(out_t[:, m, sl], z1[:, m, :],
                                        x_t[0:96, m, sl], OP.add)
            if d["fi"] == NF - 1:
                o0 = d["im"] * MAC
                nc.sync.dma_start(out=vout[d["n"], :, :, o0:o0 + MAC],
                                  in_=out_t)
                del macs[(d["n"], d["im"])]

        stage_a(0)
        stage_c(0)
        for i in range(1, len(tiles)):
            stage_a(i)
            stage_b(i - 1)
            stage_c(i)
        stage_b(len(tiles) - 1)

    return nc


def _get_program():
    if "nc" not in _prog_cache:
        nc = _build_program()
        _split_excess_waits(nc)
        _prog_cache["nc"] = nc
    return _prog_cache["nc"]


def kernel(**inputs):
    x = np.ascontiguousarray(inputs["x"], dtype=np.float32)
    x_list = np.ascontiguousarray(inputs["x_list"], dtype=np.float32)
    f_w = np.asarray(inputs["f_w"], dtype=np.float32)
    f_b = np.asarray(inputs["f_b"], dtype=np.float32)
    h_w = np.asarray(inputs["h_w"], dtype=np.float32)
    h_b = np.asarray(inputs["h_b"], dtype=np.float32)
    proj_w = np.asarray(inputs["proj_w"], dtype=np.float32)
    proj_b = np.asarray(inputs["proj_b"], dtype=np.float32)
    ln_w = np.asarray(inputs["ln_w"], dtype=np.float32)
    ln_b = np.asarray(inputs["ln_b"], dtype=np.float32)

    # host-side weight prep (tiny)
    fw_t = f_w[:C].T                                # [C,192]
    fwj = []
    for j in range(2):
        a = np.zeros((97, 192), dtype=np.float32)
        a[0:96] = fw_t[j * 96:(j + 1) * 96]
        if j == 0:
            a[96] = f_b[:C]
        fwj.append(a)
    wgj = []
    for j in range(2):
        a = np.zeros((97, 288), dtype=np.float32)
        for l in range(L):
            a[0:96, l * 96:(l + 1) * 96] = np.repeat(
                f_w[C + l, j * 96:(j + 1) * 96][:, None], 96, axis=1)
            if j == 0:
                a[96, l * 96:(l + 1) * 96] = f_b[C + l]
        wgj.append(a)
    hw_t = np.ascontiguousarray(h_w.T)              # [C,192]
    w_mu = (proj_w.sum(axis=0) / C).astype(np.float32)
    pjc_t = np.ascontiguousarray((proj_w - w_mu[None, :]).T)  # [C,192]
    mean_pb = np.float32(proj_b.mean())
    pbt = np.ascontiguousarray((proj_b - mean_pb).reshape(2, 96).T)
    lnw_r = np.ascontiguousarray(ln_w[None, :])
    hbv = np.ascontiguousarray(h_b.reshape(2, 96).T)
    lnbv = np.ascontiguousarray(ln_b.reshape(2, 96).T)

    # augmented x with ones rows at aug-channels 96 and 193
    xs = x.reshape(NCORES, NS, C, HW)
    xa = np.empty((NCORES, NS, 194, HW), dtype=np.float32)
    xa[:, :, 0:96] = xs[:, :, 0:96]
    xa[:, :, 96] = 1.0
    xa[:, :, 97:193] = xs[:, :, 96:192]
    xa[:, :, 193] = 1.0
    xls = x_list.reshape(L, NCORES, NS, C, HW)

    common = {
        "fw0": fwj[0], "fw1": fwj[1], "wg0": wgj[0], "wg1": wgj[1],
        "hw_t": hw_t, "pjc_t": pjc_t, "lnw": lnw_r,
        "oc": np.full((96, 1), 1.0 / C, dtype=np.float32),
        "hb": hbv, "pbt": pbt, "lnbc": lnbv,
    }
    in_maps = []
    for c in range(NCORES):
        m = dict(common)
        m["x"] = xa[c]
        for l in range(L):
            m[f"xl{l}"] = xls[l, c]
        in_maps.append(m)

    nc = _get_program()
    _prog_cache["in_maps"] = in_maps
    res = run_bass_kernel_spmd(nc, in_maps, core_ids=list(range(NCORES)))
    out = np.concatenate([r["out"][None] for r in res.results], axis=0)
    return out.reshape(N_FULL, C, H, W).astype(np.float32)

